# revision 30
# baseline (speedup 1.0000x reference)
"""MoE ConditionalFeedForward kernel for 8 trn2 NeuronCores.

Strategy: paired expert parallelism with uniform weight streaming.

Routing: unique (token, expert) pairs are computed once (a token whose two
slots pick the same expert is deduped) and grouped by expert on the host.
Experts are paired large-count-with-small-count; the pair of cores (2g, 2g+1)
owns the expert pair's weights split in half along the intermediate dim I.
Both cores process ALL tokens of both experts against their I-half and emit
full-D f32 partial outputs; the host sums the two partials per expert. This
balances PE work across cores: effective capacity = (capA + capB) / 2 instead
of the max expert count.

Device program (segments s = A, B with capacities Ns; D=2048, IH = I/2):
  phase 1 (per i-block): hT[i,c] = silu(w1T.x) * (w3T.x)   (PE, d on partitions)
  phase 2 (per i-block): yT[d,c] += w2[i,d] * hT[i,c]      (PE, i on partitions)

The i-blocks are processed in chunks, with phase 2 of chunk c interleaved
after phase 1 of chunk c+1. This makes the three weight streams (w1, w3 on
the sync DMA queue; w2 on the gpsimd queue) flow at a near-constant combined
~300 GB/s for the whole kernel — there is no phase transition at which a
25 MB stream has to start cold, and no prefetch burst that starves the
just-in-time stream. Cross-chunk accumulation of yT happens in an SBUF f32
accumulator via DVE adds (PSUM can only hold 4 live output tiles next to
phase 1's 4). All weights/activations stream as bf16; PSUM accumulation, the
SBUF accumulator and the partial outputs are f32, so pairing adds no
quantization error.
"""

import numpy as np
import ml_dtypes

BF16 = ml_dtypes.bfloat16

T, A, E, D, I = 1024, 2, 8, 2048, 5632
N_CORES = 8
NG = E // 2            # core pairs / expert pairs
IH = I // 2            # i-rows per core
PAIRED = True

_BUILD_CACHE = {}


def _chunks(ib, last_small=False):
    """i-block chunks: a small first chunk so the phase-2 stream's first
    tranche has an early deadline the HBM can meet, 6-block chunks after
    (chunk size bounds live w2 SBUF tiles: 3 x 2-block units). The final
    segment's last chunk is split small so the kernel's last output DMA
    trails the last matmul by as little as possible."""
    sizes = [4]
    rem = ib - 4
    while rem > 0:
        g = min(6, rem)
        sizes.append(g)
        rem -= g
    if last_small and sizes[-1] >= 6:
        g = sizes.pop()
        sizes += [g - 3, 3]
    out, b0 = [], 0
    for n in sizes:
        out.append((b0, n))
        b0 += n
    return out


def _ph1_groups(b0, nb, fast_start):
    """w1/w3 DMA groups (block runs) inside one chunk; the very first chunk
    uses 1-block groups up front so the first matmul's weights land fast."""
    sizes = [1, 1, 1, 1] if fast_start else []
    rem = nb - len(sizes)
    while rem > 0:
        g = min(2, rem)
        sizes.append(g)
        rem -= g
    out, b = [], b0
    for g in sizes:
        out.append((b, g))
        b += g
    return out


def _build(caps, ibs, d=D):
    """Build + compile the per-core program.

    caps/ibs: per-segment token capacity and i-block count. A segment is one
    expert's token set against this core's slice of that expert's weights.
    """
    key = (caps, ibs, d)
    if key in _BUILD_CACHE:
        return _BUILD_CACHE[key]

    import concourse.mybir as mybir
    import concourse.tile as tile
    from concourse import bacc

    dt = mybir.dt
    WDT = dt.bfloat16
    F32 = dt.float32

    db = d // 128          # d-chunks (contraction of phase 1)
    dcs = d // 128         # d-column blocks of the output
    cw = db * 128          # w1/w3 columns per i-block
    nseg = len(caps)
    cmx = max(caps)        # tagged tiles share one shape; slice per segment
    for cap in caps:
        assert cap % 4 == 0 and cap <= 512
    segoff = [0]
    for ib in ibs:
        segoff.append(segoff[-1] + ib)
    ib_tot = segoff[-1]

    nc = bacc.Bacc("TRN2", target_bir_lowering=False, debug=False,
                   num_devices=N_CORES)

    xs, ys = [], []
    for s, cap in enumerate(caps):
        xs.append(nc.dram_tensor(f"xgt{s}", [128, db * cap], WDT,
                                 kind="ExternalInput").ap())
        ys.append(nc.dram_tensor(f"yt{s}", [128, (d // 128) * cap], F32,
                                 kind="ExternalOutput").ap())
    w1d = nc.dram_tensor("w1d", [128, ib_tot * cw], WDT,
                         kind="ExternalInput").ap()
    w3d = nc.dram_tensor("w3d", [128, ib_tot * cw], WDT,
                         kind="ExternalInput").ap()
    w2d = nc.dram_tensor("w2d", [128, ib_tot * d], WDT,
                         kind="ExternalInput").ap()

    with tile.TileContext(nc) as tc:
        with (
            tc.tile_pool(name="xpool", bufs=1) as xpool,
            tc.tile_pool(name="w1pool", bufs=6) as w1pool,
            tc.tile_pool(name="w3pool", bufs=6) as w3pool,
            tc.tile_pool(name="w2pool", bufs=5) as w2pool,
            tc.tile_pool(name="hpool", bufs=1) as hpool,
            tc.tile_pool(name="spool", bufs=2) as spool,
            tc.tile_pool(name="apool", bufs=1) as apool,
            tc.tile_pool(name="psA", bufs=2, space="PSUM") as psA,
            tc.tile_pool(name="psB", bufs=1, space="PSUM") as psB,
        ):
            xg, h = [], []
            for s, cap in enumerate(caps):
                xg.append(xpool.tile([128, db * cap], WDT, name=f"xg{s}"))
                h.append(hpool.tile([128, ibs[s] * cap], WDT, name=f"h{s}"))

            def load_x(s, eng):
                # two big-descriptor transfers: small descriptors tank the
                # per-queue DMA rate (~22 GB/s per descriptor stream)
                cap = caps[s]
                for a, b in ((0, 4), (4, db)):
                    eng.dma_start(xg[s][:, a * cap:b * cap],
                                  xs[s][:, a * cap:b * cap])

            # segment 0's x is on the startup critical path: split it in
            # consumption order across all three queues (phase 1 contracts
            # d-chunks ascending). The sync queue's first transfers stay
            # w1/w3 group 0 (PE's first dependency); its x tail is emitted
            # inside ph1 right after them.
            t1x = (db * 5 // 8) * caps[0]
            nc.gpsimd.dma_start(xg[0][:, :t1x], xs[0][:, :t1x])
            nc.scalar.dma_start(xg[0][:, t1x:], xs[0][:, t1x:])

            acc = [None] * nseg

            def ph1(s, c):
                b0, nb = c
                cap = caps[s]
                for g0, gn in _ph1_groups(b0, nb, fast_start=(s, b0) == (0, 0)):
                    ga = segoff[s] * cw + g0 * cw
                    wt1 = w1pool.tile([128, 2 * cw], WDT, tag="w1")
                    nc.sync.dma_start(wt1[:, :gn * cw],
                                      w1d[:, ga:ga + gn * cw])
                    wt3 = w3pool.tile([128, 2 * cw], WDT, tag="w3")
                    nc.sync.dma_start(wt3[:, :gn * cw],
                                      w3d[:, ga:ga + gn * cw])
                    for si in range(gn):
                        b = g0 + si
                        ps1 = psA.tile([128, cmx], F32, tag="ps1")
                        ps3 = psA.tile([128, cmx], F32, tag="ps3")
                        for do in range(db):
                            lo = (si * db + do) * 128
                            nc.tensor.matmul(
                                ps1[:, :cap], wt1[:, lo:lo + 128],
                                xg[s][:, do * cap:(do + 1) * cap],
                                start=(do == 0), stop=(do == db - 1))
                        for do in range(db):
                            lo = (si * db + do) * 128
                            nc.tensor.matmul(
                                ps3[:, :cap], wt3[:, lo:lo + 128],
                                xg[s][:, do * cap:(do + 1) * cap],
                                start=(do == 0), stop=(do == db - 1))
                        sil = spool.tile([128, cmx], F32, tag="sil")
                        nc.scalar.activation(
                            sil[:, :cap], ps1[:, :cap],
                            mybir.ActivationFunctionType.Silu)
                        nc.vector.tensor_mul(
                            h[s][:, b * cap:(b + 1) * cap], sil[:, :cap],
                            ps3[:, :cap])

            def ph2(s, c):
                b0, nb = c
                cap = caps[s]
                first_chunk = b0 == 0
                last_chunk = b0 + nb == ibs[s]
                if first_chunk:
                    # one contiguous accumulator per segment (dc-major), so
                    # outputs leave as 4 big-descriptor DMAs per segment
                    # instead of 16 small ones that exhaust the DMA queues'
                    # completion semaphores
                    acc[s] = apool.tile([128, dcs * cap], F32,
                                        tag=f"acc{s}", name=f"acc{s}")
                # 2-block w2 units; each holds its blocks' full D columns.
                # The kernel's first two chunks use 1-block units so the w2
                # stream's initial fill ramps up instead of bursting while
                # the w1/w3 stream is still at zero lead.
                usz = 1 if (s == 0 and b0 == 0) else 2
                units = []
                u0 = 0
                while u0 < nb:
                    un = min(usz, nb - u0)
                    units.append((u0, un))
                    u0 += un
                wts = []
                for u0, un in units:
                    ga = (segoff[s] + b0 + u0) * d
                    wt2 = w2pool.tile([128, 2 * d], WDT, tag="w2")
                    nc.gpsimd.dma_start(wt2[:, :un * d],
                                        w2d[:, ga:ga + un * d])
                    wts.append(wt2)
                for dcg in range(0, dcs, 4):
                    po = {}
                    for dc in range(dcg, dcg + 4):
                        po[dc] = psB.tile([128, cmx], F32, tag=f"po{dc % 4}",
                                          name=f"po{s}_{dc}")
                    for ui, (u0, un) in enumerate(units):
                        for si in range(un):
                            b = b0 + u0 + si
                            for dc in range(dcg, dcg + 4):
                                nc.tensor.matmul(
                                    po[dc][:, :cap],
                                    wts[ui][:, si * d + dc * 128:
                                            si * d + dc * 128 + 128],
                                    h[s][:, b * cap:(b + 1) * cap],
                                    start=(b == b0), stop=(b == b0 + nb - 1))
                    for dc in range(dcg, dcg + 4):
                        asl = acc[s][:, dc * cap:(dc + 1) * cap]
                        if first_chunk:
                            nc.vector.tensor_copy(asl, po[dc][:, :cap])
                        else:
                            nc.vector.tensor_add(asl, asl, po[dc][:, :cap])
                    if last_chunk:
                        # one output DMA per dc-group straight from the
                        # persistent accumulator: big descriptors, no
                        # completion-semaphore recycling, and never on the
                        # scalar queue (where phase 1's silus live)
                        eng = (nc.sync, nc.gpsimd)[(dcg // 4) % 2]
                        eng.dma_start(
                            ys[s][:, dcg * cap:(dcg + 4) * cap],
                            acc[s][:, dcg * cap:(dcg + 4) * cap])

            # software-pipelined task order: ph2 of chunk k runs after ph1 of
            # chunk k+1, so phase 2 never waits on the silu/mul tail of its
            # own chunk, and w1/w3/w2 stream concurrently all kernel long.
            tasks = [(s, c) for s in range(nseg)
                     for c in _chunks(ibs[s], last_small=(s == nseg - 1))]
            loaded = {0}
            for k, t in enumerate(tasks):
                ph1(*t)
                # queue the next segment's x once this segment is well
                # underway (it isn't needed until that segment's phase 1,
                # and loading it at t=0 competes with the critical streams)
                if t[0] + 1 < nseg and t[1][0] > 4 and t[0] + 1 not in loaded:
                    load_x(t[0] + 1, nc.scalar)
                    loaded.add(t[0] + 1)
                if k >= 1:
                    ph2(*tasks[k - 1])
            ph2(*tasks[-1])

    nc.compile()
    _BUILD_CACHE[key] = nc
    return nc


def _pack_w13(wk, d=D):
    """[ih, d] -> [di, b, do, i_in] flattened to [128, ib*db*128]."""
    db, ib = d // 128, wk.shape[0] // 128
    return np.ascontiguousarray(
        wk.reshape(ib, 128, db, 128).transpose(3, 0, 2, 1)
    ).reshape(128, ib * db * 128)


def _pack_w2(wk, d=D):
    """[ih, d] -> [i_in, b, dcol] flattened to [128, ib*d] (block-major)."""
    ib = wk.shape[0] // 128
    return np.ascontiguousarray(
        wk.reshape(ib, 128, d).transpose(1, 0, 2)).reshape(128, ib * d)


def _prepare(inputs):
    x = np.asarray(inputs["x"])
    idx = np.asarray(inputs["expert_indices"])
    w1 = np.asarray(inputs["w1"])
    w2 = np.asarray(inputs["w2"])
    w3 = np.asarray(inputs["w3"])

    t, a = idx.shape
    d = x.shape[1]
    db = d // 128

    # ---- host routing, deduped per (token, expert) ----
    flat = idx.reshape(-1).astype(np.int64)
    code = np.repeat(np.arange(t, dtype=np.int64), a) * E + flat
    ucode = np.unique(code)
    ue = ucode % E
    order = np.argsort(ue, kind="stable")
    ucode_g = ucode[order]
    counts = np.bincount(ue, minlength=E)
    starts = np.concatenate([[0], np.cumsum(counts)])
    lut = np.full(t * E, -1, np.int64)
    lut[ucode_g] = np.arange(len(ucode_g))

    x_bf = x.astype(BF16)

    def pack_x(k, cap):
        sel = (ucode_g[starts[k]:starts[k + 1]]) // E
        xgp = np.zeros((cap, d), BF16)
        xgp[:len(sel)] = x_bf[sel]
        return np.ascontiguousarray(
            xgp.T.reshape(db, 128, cap).transpose(1, 0, 2)
        ).reshape(128, db * cap)

    def r4(n):
        return max(128, int(-(-n // 4) * 4))

    if PAIRED:
        by_count = np.argsort(-counts, kind="stable")
        pairs = [(int(by_count[g]), int(by_count[E - 1 - g]))
                 for g in range(NG)]
        caps = (r4(max(counts[p[0]] for p in pairs)),
                r4(max(counts[p[1]] for p in pairs)))
        ibs = (IH // 128, IH // 128)
        nc = _build(caps, ibs, d)
        in_maps = []
        for g in range(NG):
            ea, eb = pairs[g]
            xa, xb = pack_x(ea, caps[0]), pack_x(eb, caps[1])
            for hf in range(2):
                rows = slice(hf * IH, (hf + 1) * IH)
                in_maps.append({
                    "xgt0": xa,
                    "xgt1": xb,
                    "w1d": np.concatenate(
                        [_pack_w13(w1[ea][rows].astype(BF16), d),
                         _pack_w13(w1[eb][rows].astype(BF16), d)], axis=1),
                    "w3d": np.concatenate(
                        [_pack_w13(w3[ea][rows].astype(BF16), d),
                         _pack_w13(w3[eb][rows].astype(BF16), d)], axis=1),
                    "w2d": np.concatenate(
                        [_pack_w2(w2[ea][rows].astype(BF16), d),
                         _pack_w2(w2[eb][rows].astype(BF16), d)], axis=1),
                })
        return nc, in_maps, (t, a, d, code, lut, counts, starts, pairs)
    else:
        cap = r4(counts.max())
        nc = _build((cap,), (w1.shape[1] // 128,), d)
        in_maps = []
        for k in range(E):
            in_maps.append({
                "xgt0": pack_x(k, cap),
                "w1d": _pack_w13(w1[k].astype(BF16), d),
                "w3d": _pack_w13(w3[k].astype(BF16), d),
                "w2d": _pack_w2(w2[k].astype(BF16), d),
            })
        return nc, in_maps, (t, a, d, code, lut, counts, starts, None)


def _scatter(results, scatter_info):
    t, a, d, code, lut, counts, starts, pairs = scatter_info
    rows_by_expert = [None] * E
    def unpack(buf):
        # [128, dcs*cap] partition-major -> [d, cap]
        dcs = d // 128
        cap = buf.shape[1] // dcs
        return buf.reshape(128, dcs, cap).transpose(1, 0, 2).reshape(d, cap)

    if pairs is not None:
        for g in range(NG):
            ea, eb = pairs[g]
            ya = unpack(results[2 * g]["yt0"] + results[2 * g + 1]["yt0"])
            yb = unpack(results[2 * g]["yt1"] + results[2 * g + 1]["yt1"])
            rows_by_expert[ea] = ya[:, :counts[ea]].T
            rows_by_expert[eb] = yb[:, :counts[eb]].T
    else:
        for k in range(E):
            rows_by_expert[k] = unpack(results[k]["yt0"])[:, :counts[k]].T
    rows = np.concatenate(rows_by_expert, axis=0)
    out_flat = rows[lut[code]]
    return np.ascontiguousarray(out_flat.reshape(t, a, d), np.float32)


def kernel(**inputs):
    from concourse.bass_utils import run_bass_kernel_spmd

    nc, in_maps, scatter_info = _prepare(inputs)
    res = run_bass_kernel_spmd(nc, in_maps, core_ids=list(range(N_CORES)))
    return _scatter(res.results, scatter_info)


# revision 31
# speedup vs baseline: 1.0694x; 1.0694x over previous
"""MoE ConditionalFeedForward kernel for 8 trn2 NeuronCores.

Strategy: paired expert parallelism with uniform weight streaming.

Routing: unique (token, expert) pairs are computed once (a token whose two
slots pick the same expert is deduped) and grouped by expert on the host.
Experts are paired large-count-with-small-count; the pair of cores (2g, 2g+1)
owns the expert pair's weights split in half along the intermediate dim I.
Both cores process ALL tokens of both experts against their I-half and emit
full-D f32 partial outputs; the host sums the two partials per expert. This
balances PE work across cores: effective capacity = (capA + capB) / 2 instead
of the max expert count.

Device program (segments s = A, B with capacities Ns; D=2048, IH = I/2):
  phase 1 (per i-block): hT[i,c] = silu(w1T.x) * (w3T.x)   (PE, d on partitions)
  phase 2 (per i-block): yT[d,c] += w2[i,d] * hT[i,c]      (PE, i on partitions)

The i-blocks are processed in chunks, with phase 2 of chunk c interleaved
after phase 1 of chunk c+1. This makes the three weight streams (w1, w3 on
the sync DMA queue; w2 on the gpsimd queue) flow at a near-constant combined
~300 GB/s for the whole kernel — there is no phase transition at which a
25 MB stream has to start cold, and no prefetch burst that starves the
just-in-time stream. Cross-chunk accumulation of yT happens in an SBUF f32
accumulator via DVE adds (PSUM can only hold 4 live output tiles next to
phase 1's 4). All weights/activations stream as bf16; PSUM accumulation, the
SBUF accumulator and the partial outputs are f32, so pairing adds no
quantization error.
"""

import numpy as np
import ml_dtypes

BF16 = ml_dtypes.bfloat16

T, A, E, D, I = 1024, 2, 8, 2048, 5632
N_CORES = 8
NG = E // 2            # core pairs / expert pairs
IH = I // 2            # i-rows per core
PAIRED = True

_BUILD_CACHE = {}


def _chunks(ib, last_small=False):
    """i-block chunks: a small first chunk so the phase-2 stream's first
    tranche has an early deadline the HBM can meet, 6-block chunks after
    (chunk size bounds live w2 SBUF tiles: 3 x 2-block units). The final
    segment's last chunk is split small so the kernel's last output DMA
    trails the last matmul by as little as possible."""
    sizes = [4]
    rem = ib - 4
    while rem > 0:
        g = min(6, rem)
        sizes.append(g)
        rem -= g
    if last_small and sizes[-1] >= 6:
        g = sizes.pop()
        sizes += [g - 3, 3]
    out, b0 = [], 0
    for n in sizes:
        out.append((b0, n))
        b0 += n
    return out


def _ph1_groups(b0, nb, fast_start):
    """w1/w3 DMA groups (block runs) inside one chunk; the very first chunk
    uses 1-block groups up front so the first matmul's weights land fast."""
    sizes = [1, 1] if fast_start else []
    rem = nb - len(sizes)
    while rem > 0:
        g = min(2, rem)
        sizes.append(g)
        rem -= g
    out, b = [], b0
    for g in sizes:
        out.append((b, g))
        b += g
    return out


def _build(caps, ibs, d=D):
    """Build + compile the per-core program.

    caps/ibs: per-segment token capacity and i-block count. A segment is one
    expert's token set against this core's slice of that expert's weights.
    """
    key = (caps, ibs, d)
    if key in _BUILD_CACHE:
        return _BUILD_CACHE[key]

    import concourse.mybir as mybir
    import concourse.tile as tile
    from concourse import bacc

    dt = mybir.dt
    WDT = dt.bfloat16
    F32 = dt.float32

    db = d // 128          # d-chunks (contraction of phase 1)
    dcs = d // 128         # d-column blocks of the output
    cw = db * 128          # w1/w3 columns per i-block
    nseg = len(caps)
    cmx = max(caps)        # tagged tiles share one shape; slice per segment
    for cap in caps:
        assert cap % 4 == 0 and cap <= 512
    segoff = [0]
    for ib in ibs:
        segoff.append(segoff[-1] + ib)
    ib_tot = segoff[-1]

    nc = bacc.Bacc("TRN2", target_bir_lowering=False, debug=False,
                   num_devices=N_CORES)

    xs, ys = [], []
    for s, cap in enumerate(caps):
        xs.append(nc.dram_tensor(f"xgt{s}", [128, db * cap], WDT,
                                 kind="ExternalInput").ap())
        ys.append(nc.dram_tensor(f"yt{s}", [128, (d // 128) * cap], F32,
                                 kind="ExternalOutput").ap())
    w1d = nc.dram_tensor("w1d", [128, ib_tot * cw], WDT,
                         kind="ExternalInput").ap()
    w3d = nc.dram_tensor("w3d", [128, ib_tot * cw], WDT,
                         kind="ExternalInput").ap()
    w2d = nc.dram_tensor("w2d", [128, ib_tot * d], WDT,
                         kind="ExternalInput").ap()

    with tile.TileContext(nc) as tc:
        with (
            tc.tile_pool(name="xpool", bufs=1) as xpool,
            tc.tile_pool(name="w1pool", bufs=6) as w1pool,
            tc.tile_pool(name="w3pool", bufs=6) as w3pool,
            tc.tile_pool(name="w2pool", bufs=5) as w2pool,
            tc.tile_pool(name="hpool", bufs=1) as hpool,
            tc.tile_pool(name="spool", bufs=2) as spool,
            tc.tile_pool(name="apool", bufs=1) as apool,
            tc.tile_pool(name="psA", bufs=2, space="PSUM") as psA,
            tc.tile_pool(name="psB", bufs=1, space="PSUM") as psB,
        ):
            xg, h = [], []
            for s, cap in enumerate(caps):
                xg.append(xpool.tile([128, db * cap], WDT, name=f"xg{s}"))
                h.append(hpool.tile([128, ibs[s] * cap], WDT, name=f"h{s}"))

            def load_x(s, eng):
                # two big-descriptor transfers: small descriptors tank the
                # per-queue DMA rate (~22 GB/s per descriptor stream)
                cap = caps[s]
                for a, b in ((0, 4), (4, db)):
                    eng.dma_start(xg[s][:, a * cap:b * cap],
                                  xs[s][:, a * cap:b * cap])

            # segment 0's x is on the startup critical path: split it in
            # consumption order across all three queues (phase 1 contracts
            # d-chunks ascending). The sync queue's first transfers stay
            # w1/w3 group 0 (PE's first dependency); its x tail is emitted
            # inside ph1 right after them.
            t1x = (db * 3 // 8) * caps[0]
            t2x = (db * 3 // 4) * caps[0]
            nc.gpsimd.dma_start(xg[0][:, :t1x], xs[0][:, :t1x])
            nc.scalar.dma_start(xg[0][:, t1x:t2x], xs[0][:, t1x:t2x])

            acc = [None] * nseg

            def ph1(s, c):
                b0, nb = c
                cap = caps[s]
                for g0, gn in _ph1_groups(b0, nb, fast_start=(s, b0) == (0, 0)):
                    ga = segoff[s] * cw + g0 * cw
                    wt1 = w1pool.tile([128, 2 * cw], WDT, tag="w1")
                    nc.sync.dma_start(wt1[:, :gn * cw],
                                      w1d[:, ga:ga + gn * cw])
                    wt3 = w3pool.tile([128, 2 * cw], WDT, tag="w3")
                    nc.sync.dma_start(wt3[:, :gn * cw],
                                      w3d[:, ga:ga + gn * cw])
                    if (s, g0) == (0, 0):
                        t2x = (db * 3 // 4) * caps[0]
                        nc.sync.dma_start(xg[0][:, t2x:], xs[0][:, t2x:])
                    for si in range(gn):
                        b = g0 + si
                        ps1 = psA.tile([128, cmx], F32, tag="ps1")
                        ps3 = psA.tile([128, cmx], F32, tag="ps3")
                        for do in range(db):
                            lo = (si * db + do) * 128
                            nc.tensor.matmul(
                                ps1[:, :cap], wt1[:, lo:lo + 128],
                                xg[s][:, do * cap:(do + 1) * cap],
                                start=(do == 0), stop=(do == db - 1))
                        for do in range(db):
                            lo = (si * db + do) * 128
                            nc.tensor.matmul(
                                ps3[:, :cap], wt3[:, lo:lo + 128],
                                xg[s][:, do * cap:(do + 1) * cap],
                                start=(do == 0), stop=(do == db - 1))
                        sil = spool.tile([128, cmx], F32, tag="sil")
                        nc.scalar.activation(
                            sil[:, :cap], ps1[:, :cap],
                            mybir.ActivationFunctionType.Silu)
                        nc.vector.tensor_mul(
                            h[s][:, b * cap:(b + 1) * cap], sil[:, :cap],
                            ps3[:, :cap])

            def ph2(s, c):
                b0, nb = c
                cap = caps[s]
                first_chunk = b0 == 0
                last_chunk = b0 + nb == ibs[s]
                if first_chunk:
                    # one contiguous accumulator per segment (dc-major), so
                    # outputs leave as 4 big-descriptor DMAs per segment
                    # instead of 16 small ones that exhaust the DMA queues'
                    # completion semaphores
                    acc[s] = apool.tile([128, dcs * cap], F32,
                                        tag=f"acc{s}", name=f"acc{s}")
                # 2-block w2 units; each holds its blocks' full D columns.
                # The kernel's first two chunks use 1-block units so the w2
                # stream's initial fill ramps up instead of bursting while
                # the w1/w3 stream is still at zero lead.
                usz = 1 if (s == 0 and b0 == 0) else 2
                units = []
                u0 = 0
                while u0 < nb:
                    un = min(usz, nb - u0)
                    units.append((u0, un))
                    u0 += un
                wts = []
                for u0, un in units:
                    ga = (segoff[s] + b0 + u0) * d
                    wt2 = w2pool.tile([128, 2 * d], WDT, tag="w2")
                    nc.gpsimd.dma_start(wt2[:, :un * d],
                                        w2d[:, ga:ga + un * d])
                    wts.append(wt2)
                for dcg in range(0, dcs, 4):
                    po = {}
                    for dc in range(dcg, dcg + 4):
                        po[dc] = psB.tile([128, cmx], F32, tag=f"po{dc % 4}",
                                          name=f"po{s}_{dc}")
                    for ui, (u0, un) in enumerate(units):
                        for si in range(un):
                            b = b0 + u0 + si
                            for dc in range(dcg, dcg + 4):
                                nc.tensor.matmul(
                                    po[dc][:, :cap],
                                    wts[ui][:, si * d + dc * 128:
                                            si * d + dc * 128 + 128],
                                    h[s][:, b * cap:(b + 1) * cap],
                                    start=(b == b0), stop=(b == b0 + nb - 1))
                    for dc in range(dcg, dcg + 4):
                        asl = acc[s][:, dc * cap:(dc + 1) * cap]
                        if first_chunk:
                            nc.vector.tensor_copy(asl, po[dc][:, :cap])
                        else:
                            nc.vector.tensor_add(asl, asl, po[dc][:, :cap])
                    if last_chunk:
                        # one output DMA per dc-group straight from the
                        # persistent accumulator: big descriptors, no
                        # completion-semaphore recycling, and never on the
                        # scalar queue (where phase 1's silus live)
                        eng = (nc.sync, nc.gpsimd)[(dcg // 4) % 2]
                        eng.dma_start(
                            ys[s][:, dcg * cap:(dcg + 4) * cap],
                            acc[s][:, dcg * cap:(dcg + 4) * cap])

            # software-pipelined task order: ph2 of chunk k runs after ph1 of
            # chunk k+1, so phase 2 never waits on the silu/mul tail of its
            # own chunk, and w1/w3/w2 stream concurrently all kernel long.
            tasks = [(s, c) for s in range(nseg)
                     for c in _chunks(ibs[s], last_small=(s == nseg - 1))]
            loaded = {0}
            for k, t in enumerate(tasks):
                ph1(*t)
                # queue the next segment's x once this segment is well
                # underway (it isn't needed until that segment's phase 1,
                # and loading it at t=0 competes with the critical streams)
                if t[0] + 1 < nseg and t[1][0] > 4 and t[0] + 1 not in loaded:
                    load_x(t[0] + 1, nc.scalar)
                    loaded.add(t[0] + 1)
                if k >= 1:
                    ph2(*tasks[k - 1])
            ph2(*tasks[-1])

    nc.compile()
    _BUILD_CACHE[key] = nc
    return nc


def _pack_w13(wk, d=D):
    """[ih, d] -> [di, b, do, i_in] flattened to [128, ib*db*128]."""
    db, ib = d // 128, wk.shape[0] // 128
    return np.ascontiguousarray(
        wk.reshape(ib, 128, db, 128).transpose(3, 0, 2, 1)
    ).reshape(128, ib * db * 128)


def _pack_w2(wk, d=D):
    """[ih, d] -> [i_in, b, dcol] flattened to [128, ib*d] (block-major)."""
    ib = wk.shape[0] // 128
    return np.ascontiguousarray(
        wk.reshape(ib, 128, d).transpose(1, 0, 2)).reshape(128, ib * d)


def _prepare(inputs):
    x = np.asarray(inputs["x"])
    idx = np.asarray(inputs["expert_indices"])
    w1 = np.asarray(inputs["w1"])
    w2 = np.asarray(inputs["w2"])
    w3 = np.asarray(inputs["w3"])

    t, a = idx.shape
    d = x.shape[1]
    db = d // 128

    # ---- host routing, deduped per (token, expert) ----
    flat = idx.reshape(-1).astype(np.int64)
    code = np.repeat(np.arange(t, dtype=np.int64), a) * E + flat
    ucode = np.unique(code)
    ue = ucode % E
    order = np.argsort(ue, kind="stable")
    ucode_g = ucode[order]
    counts = np.bincount(ue, minlength=E)
    starts = np.concatenate([[0], np.cumsum(counts)])
    lut = np.full(t * E, -1, np.int64)
    lut[ucode_g] = np.arange(len(ucode_g))

    x_bf = x.astype(BF16)

    def pack_x(k, cap):
        sel = (ucode_g[starts[k]:starts[k + 1]]) // E
        xgp = np.zeros((cap, d), BF16)
        xgp[:len(sel)] = x_bf[sel]
        return np.ascontiguousarray(
            xgp.T.reshape(db, 128, cap).transpose(1, 0, 2)
        ).reshape(128, db * cap)

    def r4(n):
        return max(128, int(-(-n // 4) * 4))

    if PAIRED:
        by_count = np.argsort(-counts, kind="stable")
        pairs = [(int(by_count[g]), int(by_count[E - 1 - g]))
                 for g in range(NG)]
        caps = (r4(max(counts[p[0]] for p in pairs)),
                r4(max(counts[p[1]] for p in pairs)))
        ibs = (IH // 128, IH // 128)
        nc = _build(caps, ibs, d)
        in_maps = []
        for g in range(NG):
            ea, eb = pairs[g]
            xa, xb = pack_x(ea, caps[0]), pack_x(eb, caps[1])
            for hf in range(2):
                rows = slice(hf * IH, (hf + 1) * IH)
                in_maps.append({
                    "xgt0": xa,
                    "xgt1": xb,
                    "w1d": np.concatenate(
                        [_pack_w13(w1[ea][rows].astype(BF16), d),
                         _pack_w13(w1[eb][rows].astype(BF16), d)], axis=1),
                    "w3d": np.concatenate(
                        [_pack_w13(w3[ea][rows].astype(BF16), d),
                         _pack_w13(w3[eb][rows].astype(BF16), d)], axis=1),
                    "w2d": np.concatenate(
                        [_pack_w2(w2[ea][rows].astype(BF16), d),
                         _pack_w2(w2[eb][rows].astype(BF16), d)], axis=1),
                })
        return nc, in_maps, (t, a, d, code, lut, counts, starts, pairs)
    else:
        cap = r4(counts.max())
        nc = _build((cap,), (w1.shape[1] // 128,), d)
        in_maps = []
        for k in range(E):
            in_maps.append({
                "xgt0": pack_x(k, cap),
                "w1d": _pack_w13(w1[k].astype(BF16), d),
                "w3d": _pack_w13(w3[k].astype(BF16), d),
                "w2d": _pack_w2(w2[k].astype(BF16), d),
            })
        return nc, in_maps, (t, a, d, code, lut, counts, starts, None)


def _scatter(results, scatter_info):
    t, a, d, code, lut, counts, starts, pairs = scatter_info
    rows_by_expert = [None] * E
    def unpack(buf):
        # [128, dcs*cap] partition-major -> [d, cap]
        dcs = d // 128
        cap = buf.shape[1] // dcs
        return buf.reshape(128, dcs, cap).transpose(1, 0, 2).reshape(d, cap)

    if pairs is not None:
        for g in range(NG):
            ea, eb = pairs[g]
            ya = unpack(results[2 * g]["yt0"] + results[2 * g + 1]["yt0"])
            yb = unpack(results[2 * g]["yt1"] + results[2 * g + 1]["yt1"])
            rows_by_expert[ea] = ya[:, :counts[ea]].T
            rows_by_expert[eb] = yb[:, :counts[eb]].T
    else:
        for k in range(E):
            rows_by_expert[k] = unpack(results[k]["yt0"])[:, :counts[k]].T
    rows = np.concatenate(rows_by_expert, axis=0)
    out_flat = rows[lut[code]]
    return np.ascontiguousarray(out_flat.reshape(t, a, d), np.float32)


def kernel(**inputs):
    from concourse.bass_utils import run_bass_kernel_spmd

    nc, in_maps, scatter_info = _prepare(inputs)
    res = run_bass_kernel_spmd(nc, in_maps, core_ids=list(range(N_CORES)))
    return _scatter(res.results, scatter_info)
